# revision 2
# baseline (speedup 1.0000x reference)
"""Trainium2 Bass kernel for dual-score multi-head self-attention.

Reference computation (per batch b):
    q   = x @ Wq.T + bq          k = x @ Wk.T + bk
    v   = x @ Wv.T + bv          pos = pos_emb @ Wp.T + bp
    scores = (q k^T + q pos^T) / sqrt(dh)   (per head)
    out = softmax(scores) @ v, merged heads, @ Wo.T + bo

Algebraic folds:
  * q k^T + q pos^T == q (k+pos)^T; k+pos accumulates in one PSUM tile.
  * the k-side bias (bk+bp) only shifts each query's score row by a
    constant (q . bkp), which softmax cancels -> dropped entirely.
  * v's bias commutes through softmax (rows sum to 1): bo' = bo + Wo @ bv.
  * softmax denominators ride along in the attention*V matmul via ones
    columns appended to V (even heads [v|ones], odd heads [ones|v] so
    every DVE op in the normalization runs at a legal base partition).

fp8: x/pos and the q/k/p/v projection weights are fp8(e4m3); the q/k/v
projections and the attention*V matmuls run in DoubleRow perf mode
(2 fp8 contraction rows per PE cell). Exp tiles are produced directly
in fp8: ScalarE activations write e4m3, DVE tiles use a Schraudolph
bit trick into the raw e4m3 bytes. Scores stay bf16 (K=64 pairs already
overlap via row tiling), the out projection stays bf16 for accuracy.

Sharding: batch parallel, one batch per core, weights replicated, no
collectives. See bottom for host-side prep.
"""

import numpy as np

try:
    import concourse.bass as bass  # noqa: F401
except ImportError:  # pragma: no cover - container default path
    import sys

    for p in ("/opt/trn_rl_repo", "/root/.axon_site/_ro/trn_rl_repo"):
        if p not in sys.path:
            sys.path.insert(0, p)

import ml_dtypes

import concourse.bass as bass
import concourse.mybir as mybir
import concourse.tile as tile
from concourse import bacc
from concourse.bass import ts
from concourse.bass_utils import run_bass_kernel_spmd

P = 128
T = 2048
D = 512
H = 8
DH = 64
B = 8
MT = D // P  # 4 feature tiles
KT = T // P  # 16 kpos tiles
KTP = KT // 2  # 8 key-tile PAIRS (DoubleRow granularity)
QC = 512  # q chunk (one PSUM bank of f32)
NQC = T // QC

BF = mybir.dt.bfloat16
F32 = mybir.dt.float32
FP8 = mybir.dt.float8e4
U8 = mybir.dt.uint8
ADD = mybir.AluOpType.add
MULT = mybir.AluOpType.mult
EXP = mybir.ActivationFunctionType.Exp
DR = mybir.MatmulPerfMode.DoubleRow

N_CORES = 8

# kt indices whose exp tile is computed on VectorE via the Schraudolph
# fp8-bit trick instead of ScalarE (load balancing; [] = all on ScalarE).
SCHRAUDOLPH_KTS = (1, 3, 7, 9, 13, 15)
# e4m3 bits(exp(s/8)) ~= round(A*s + B): 8 codes/octave, bias 7.
_SCH_A = 8.0 * 1.4426950408889634 * 0.125
_SCH_B = 8.0 * (7.0 - 0.043677448)


def _emit(ctx, tc, io):
    nc = tc.nc

    # ---- persistent SBUF tensors -------------------------------------
    const_pool = ctx.enter_context(tc.tile_pool(name="const", bufs=1))

    def single(name, shape, dtype):
        return const_pool.tile(shape, dtype, name=name, tag=name)

    w_sb = {w: single(f"w_{w}", [P, MT, D], FP8) for w in
            ("wq", "wk", "wp", "wv")}
    w_sb["wo"] = single("w_wo", [P, MT, D], BF)
    b_sb = {b: single(f"b_{b}", [P, MT], F32) for b in ("bq", "bob")}
    xT_sb = single("xT_sb", [P, MT, T], FP8)
    posT_sb = single("posT_sb", [P, MT, T], FP8)
    qT_sb = [single(f"qT{m}", [P, T], BF) for m in range(MT)]
    kT_sb = [single(f"kT{m}", [P, T], BF) for m in range(MT)]
    ctx_sb = [single(f"ctxT{m}", [P, T], BF) for m in range(MT)]
    # v for kt pair ktp: [:, i, h, :] = v(+ones) of head h, key tile 2*ktp+i
    v_sb = [single(f"v{ktp}", [P, 2, H, P], FP8) for ktp in range(KTP)]

    # ---- input DMAs: few big transfers, ordered so compute starts
    # early; alternate the two HWDGE queues (SP + ACT; ACT only issues
    # DMAs during the load phase, before exps start).
    qs = [nc.sync, nc.scalar]

    def loadw(w, qi):
        for k in range(MT):
            qs[(qi + k) % 2].dma_start(out=w_sb[w][:, k, :], in_=io[w + "T"][k])

    def loadb(b):
        for m in range(MT):
            nc.scalar.dma_start(out=b_sb[b][:, m : m + 1], in_=io[b][m])

    def loadx(sb, dram, lo, hi, qi):
        for k in range(MT):
            qs[(qi + k) % 2].dma_start(out=sb[:, k, lo:hi], in_=dram[k][:, lo:hi])

    loadw("wq", 0)
    loadx(xT_sb, io["xT"], 0, QC, 1)
    loadw("wk", 0)
    loadx(posT_sb, io["posT"], 0, QC, 1)
    loadw("wp", 0)
    loadb("bq")
    loadw("wv", 0)
    loadx(xT_sb, io["xT"], QC, 2 * QC, 0)
    loadx(xT_sb, io["xT"], 2 * QC, 3 * QC, 1)
    loadx(xT_sb, io["xT"], 3 * QC, T, 0)
    loadx(posT_sb, io["posT"], QC, 2 * QC, 1)
    loadx(posT_sb, io["posT"], 2 * QC, 3 * QC, 0)
    loadx(posT_sb, io["posT"], 3 * QC, T, 1)
    loadw("wo", 0)
    loadb("bob")

    for ktp in range(KTP):
        # only the ones-regions: keeps the v drains (disjoint slices)
        # independent of the memsets under subtile dep tracking; gpsimd
        # keeps them off the DVE queue during the pipeline fill
        nc.gpsimd.memset(v_sb[ktp][:, :, 0 : H : 2, DH:P], 1.0)
        nc.gpsimd.memset(v_sb[ktp][:, :, 1 : H : 2, 0:DH], 1.0)

    # ---- pools --------------------------------------------------------
    ps_proj = ctx.enter_context(tc.tile_pool(name="ps_proj", bufs=2, space="PSUM"))
    ps_sc = ctx.enter_context(tc.tile_pool(name="ps_sc", bufs=2, space="PSUM"))
    ps_av = ctx.enter_context(tc.tile_pool(name="ps_av", bufs=2, space="PSUM"))
    expp = ctx.enter_context(tc.tile_pool(name="expp", bufs=6))
    recp = ctx.enter_context(tc.tile_pool(name="recp", bufs=6))
    stagep = ctx.enter_context(tc.tile_pool(name="stagep", bufs=6))
    outp = ctx.enter_context(tc.tile_pool(name="outp", bufs=4))

    # prime the ScalarE exp table set (~2.7us one-time load) during the
    # DMA phase instead of on the first real score tile
    warm = recp.tile([P, 1], F32, name="warm", tag="warm")
    nc.vector.memset(warm, 0.0)
    nc.scalar.activation(out=warm, in_=warm, func=EXP, scale=1.0)

    # ---- v projection (natural layout, no bias) ----------------------
    def emit_v_proj(tts=None):
        for tt in tts if tts is not None else range(KT):
            ps = ps_proj.tile([P, D], F32, name=f"vps{tt}", tag="ps_proj")
            for t in range(MT // 2):
                nc.tensor.matmul(
                    ps,
                    lhsT=xT_sb[:, 2 * t : 2 * t + 2, ts(tt, P)],
                    rhs=w_sb["wv"][:, 2 * t : 2 * t + 2, :],
                    start=(t == 0),
                    stop=(t == MT // 2 - 1),
                    perf_mode=DR,
                )
            ps_h = ps.rearrange("p (h d) -> p h d", h=H)
            dst = v_sb[tt // 2][:, tt % 2]
            nc.vector.tensor_copy(out=dst[:, 0:H:2, 0:DH], in_=ps_h[:, 0:H:2, :])
            nc.vector.tensor_copy(out=dst[:, 1:H:2, DH:P], in_=ps_h[:, 1:H:2, :])

    # ---- q / k' projections for one feature tile m -------------------
    def emit_qk_proj(m, qcs=None):
        for qc in qcs if qcs is not None else range(NQC):
            ps = ps_proj.tile([P, QC], F32, name=f"qps{m}_{qc}", tag="ps_proj")
            for t in range(MT // 2):
                nc.tensor.matmul(
                    ps,
                    lhsT=w_sb["wq"][:, 2 * t : 2 * t + 2, ts(m, P)],
                    rhs=xT_sb[:, 2 * t : 2 * t + 2, ts(qc, QC)],
                    start=(t == 0),
                    stop=(t == MT // 2 - 1),
                    perf_mode=DR,
                )
            nc.vector.tensor_scalar(
                out=qT_sb[m][:, ts(qc, QC)],
                in0=ps,
                scalar1=b_sb["bq"][:, m : m + 1],
                scalar2=None,
                op0=ADD,
            )
            ps2 = ps_proj.tile([P, QC], F32, name=f"kps{m}_{qc}", tag="ps_proj")
            for t in range(MT // 2):
                nc.tensor.matmul(
                    ps2,
                    lhsT=w_sb["wk"][:, 2 * t : 2 * t + 2, ts(m, P)],
                    rhs=xT_sb[:, 2 * t : 2 * t + 2, ts(qc, QC)],
                    start=(t == 0),
                    stop=False,
                    perf_mode=DR,
                )
            for t in range(MT // 2):
                nc.tensor.matmul(
                    ps2,
                    lhsT=w_sb["wp"][:, 2 * t : 2 * t + 2, ts(m, P)],
                    rhs=posT_sb[:, 2 * t : 2 * t + 2, ts(qc, QC)],
                    start=False,
                    stop=(t == MT // 2 - 1),
                    perf_mode=DR,
                )
            nc.vector.tensor_copy(out=kT_sb[m][:, ts(qc, QC)], in_=ps2)

    # ---- attention for head pair p (heads 2p, 2p+1) ------------------
    def emit_attention(p, on_qc_done=None, pre_kt=None):
        for qc in range(NQC):
            psA = ps_av.tile([P, QC], F32, name=f"avA{p}_{qc}", tag="av")
            psB = ps_av.tile([P, QC], F32, name=f"avB{p}_{qc}", tag="av")
            for ktp in range(KTP):
                # e_pair[:, i, 0:QC] = head-A exps of kt=2*ktp+i, [:, i, QC:] = head B
                e_pair = expp.tile(
                    [P, 2, 2 * QC], FP8, name=f"e{p}_{qc}_{ktp}", tag="exp"
                )
                for i in range(2):
                    kt = 2 * ktp + i
                    if pre_kt is not None:
                        pre_kt(qc, kt)
                    s_ps = ps_sc.tile(
                        [P, 2 * QC], F32, name=f"sc{p}_{qc}_{kt}", tag="sc"
                    )
                    sA, sB = s_ps[:, 0:QC], s_ps[:, QC : 2 * QC]
                    # scores^T = k'^T.T @ q^T; the two heads' K=64 matmuls
                    # run concurrently via implicit PE row tiling (0 / 64)
                    nc.tensor.matmul(
                        sA,
                        lhsT=kT_sb[p][0:DH, ts(kt, P)],
                        rhs=qT_sb[p][0:DH, ts(qc, QC)],
                        start=True,
                        stop=True,
                    )
                    nc.tensor.matmul(
                        sB,
                        lhsT=kT_sb[p][DH:P, ts(kt, P)],
                        rhs=qT_sb[p][DH:P, ts(qc, QC)],
                        start=True,
                        stop=True,
                    )
                    e_out = e_pair[:, i, :]
                    if kt in SCHRAUDOLPH_KTS or (
                        p == MT - 1 and qc == NQC - 1 and kt >= KT - 2
                    ):
                        # last two exp tiles gate the whole tail; DVE's queue
                        # is empty by then while ScalarE still drains a backlog
                        nc.vector.tensor_scalar(
                            out=e_out.bitcast(U8),
                            in0=s_ps,
                            scalar1=_SCH_A,
                            scalar2=_SCH_B,
                            op0=MULT,
                            op1=ADD,
                        )
                    else:
                        nc.scalar.activation(out=e_out, in_=s_ps, func=EXP, scale=0.125)
                # attn @ V for the kt pair, 2 fp8 rows per PE cell
                nc.tensor.matmul(
                    psA,
                    lhsT=v_sb[ktp][:, :, 2 * p, :],
                    rhs=e_pair[:, :, 0:QC],
                    start=(ktp == 0),
                    stop=(ktp == KTP - 1),
                    perf_mode=DR,
                    skip_group_check=True,
                )
                nc.tensor.matmul(
                    psB,
                    lhsT=v_sb[ktp][:, :, 2 * p + 1, :],
                    rhs=e_pair[:, :, QC : 2 * QC],
                    start=(ktp == 0),
                    stop=(ktp == KTP - 1),
                    perf_mode=DR,
                    skip_group_check=True,
                )
            # fast evacuation: free the PSUM pair with two plain copies so
            # the next q-chunk's AV matmuls never head-of-line-block PE.
            stA = stagep.tile([P, QC], F32, name=f"stA{p}_{qc}", tag="st")
            nc.vector.tensor_copy(out=stA, in_=psA)
            stB = stagep.tile([P, QC], F32, name=f"stB{p}_{qc}", tag="st")
            nc.vector.tensor_copy(out=stB, in_=psB)
            # stA = [ctxA @0:64 | denA @64:128]; stB = [denB @0:64 | ctxB @64:128]
            den = recp.tile([P, QC], F32, name=f"den{p}_{qc}", tag="den")
            nc.sync.dma_start(out=den[0:DH], in_=stA[DH:P])
            nc.sync.dma_start(out=den[DH:P], in_=stB[0:DH])
            rec = recp.tile([P, QC], F32, name=f"rec{p}_{qc}", tag="rec")
            nc.vector.reciprocal_approx_fast(out=rec, in_=den)
            # normalization muls are pure SBUF work: offload to GpSimd,
            # except the final pair whose ctx gates the kernel tail.
            eng = nc.vector if p == MT - 1 else nc.gpsimd
            eng.tensor_mul(
                out=ctx_sb[p][0:DH, ts(qc, QC)], in0=stA[0:DH], in1=rec[0:DH]
            )
            eng.tensor_mul(
                out=ctx_sb[p][DH:P, ts(qc, QC)], in0=stB[DH:P], in1=rec[DH:P]
            )
            if on_qc_done is not None:
                on_qc_done(qc)

    # ---- output projection -------------------------------------------
    part_sb = {}

    def emit_out_proj_partial(qc):
        # contract pairs 0..2 for this q-chunk early (their ctx tiles are
        # done long before pair 3); the tail then needs only the k=3 matmul
        for m in range(MT):
            ps = ps_proj.tile([P, QC], F32, name=f"pps{m}_{qc}", tag="ps_proj")
            for k in range(MT - 1):
                nc.tensor.matmul(
                    ps,
                    lhsT=w_sb["wo"][:, k, ts(m, P)],
                    rhs=ctx_sb[k][:, ts(qc, QC)],
                    start=(k == 0),
                    stop=(k == MT - 2),
                )
            pt = outp.tile([P, QC], F32, name=f"part{m}_{qc}", tag="part")
            nc.vector.tensor_copy(out=pt, in_=ps)
            part_sb[m] = pt

    def emit_out_proj_final(qc):
        for m in range(MT):
            ps = ps_proj.tile([P, QC], F32, name=f"ofin{m}_{qc}", tag="ps_proj")
            nc.tensor.matmul(
                ps,
                lhsT=w_sb["wo"][:, MT - 1, ts(m, P)],
                rhs=ctx_sb[MT - 1][:, ts(qc, QC)],
                start=True,
                stop=True,
            )
            o_sb = outp.tile([P, QC], F32, name=f"of{m}_{qc}", tag="out")
            nc.vector.scalar_tensor_tensor(
                out=o_sb,
                in0=ps,
                scalar=b_sb["bob"][:, m : m + 1],
                in1=part_sb[m],
                op0=ADD,
                op1=ADD,
            )
            nc.sync.dma_start(out=io["outT"][m][:, ts(qc, QC)], in_=o_sb)

    def emit_out_proj(qc):
        for m in range(MT):
            ps = ps_proj.tile([P, QC], F32, name=f"ops{m}_{qc}", tag="ps_proj")
            for k in range(MT):
                nc.tensor.matmul(
                    ps,
                    lhsT=w_sb["wo"][:, k, ts(m, P)],
                    rhs=ctx_sb[k][:, ts(qc, QC)],
                    start=(k == 0),
                    stop=(k == MT - 1),
                )
            o_sb = outp.tile([P, QC], F32, name=f"o{m}_{qc}", tag="out")
            nc.vector.tensor_scalar(
                out=o_sb,
                in0=ps,
                scalar1=b_sb["bob"][:, m : m + 1],
                scalar2=None,
                op0=ADD,
            )
            nc.sync.dma_start(out=io["outT"][m][:, ts(qc, QC)], in_=o_sb)

    # emission order: attention (exp-bound) starts as early as possible;
    # remaining projections backfill TensorE while ScalarE/DVE stream exps.
    emit_qk_proj(0, qcs=[0])
    emit_v_proj(tts=range(0, 2))

    def p0_hook(qc, kt):
        # pair-0 runs while inputs still stream in: emit the remaining
        # projections just-in-time so early scores/exps aren't scheduled
        # behind load-gated work.
        if qc != 0:
            return
        tt = kt + 2
        if tt < KT:
            emit_v_proj(tts=[tt])
        if kt == 1:
            emit_qk_proj(0, qcs=[1])
        if kt == 5:
            emit_qk_proj(0, qcs=[2])
        if kt == 9:
            emit_qk_proj(0, qcs=[3])

    emit_attention(0, pre_kt=p0_hook)
    for m in range(1, MT - 1):
        emit_qk_proj(m)
        emit_attention(m)
    emit_qk_proj(MT - 1)

    def out_proj_lagged(qc):
        if qc > 0:
            emit_out_proj(qc - 1)

    def p3_pre(qc, kt):
        # pre-contract pairs 0..2 of the final q-chunk mid-loop so the tail
        # only needs the k=3 matmuls after pair 3's normalization lands
        if qc == NQC - 1 and kt == 10:
            emit_out_proj_partial(qc)

    emit_attention(MT - 1, on_qc_done=out_proj_lagged, pre_kt=p3_pre)
    emit_out_proj_final(NQC - 1)


_CACHED_NC = None


def build_nc():
    global _CACHED_NC
    if _CACHED_NC is not None:
        return _CACHED_NC
    nc = bacc.Bacc("TRN2", target_bir_lowering=False, debug=False, num_devices=N_CORES)
    io = {}
    io["xT"] = nc.dram_tensor("xT", [MT, P, T], FP8, kind="ExternalInput").ap()
    io["posT"] = nc.dram_tensor("posT", [MT, P, T], FP8, kind="ExternalInput").ap()
    for wname in ("wq", "wk", "wp", "wv"):
        io[wname + "T"] = nc.dram_tensor(
            wname + "T", [MT, P, D], FP8, kind="ExternalInput"
        ).ap()
    io["woT"] = nc.dram_tensor("woT", [MT, P, D], BF, kind="ExternalInput").ap()
    for bname in ("bq", "bob"):
        io[bname] = nc.dram_tensor(bname, [MT, P, 1], F32, kind="ExternalInput").ap()
    io["outT"] = nc.dram_tensor("outT", [MT, P, T], F32, kind="ExternalOutput").ap()

    from contextlib import ExitStack

    with tile.TileContext(nc) as tc, ExitStack() as ctx:
        _emit(ctx, tc, io)
    nc.compile()
    _CACHED_NC = nc
    return nc


def _to_bf16(a):
    return np.asarray(a, dtype=np.float32).astype(ml_dtypes.bfloat16)


def _to_fp8(a):
    return np.asarray(a, dtype=np.float32).astype(ml_dtypes.float8_e4m3)


def make_in_maps(x, pos_embeddings, Wq, bq, Wk, bk, Wv, bv, Wp, bp, Wo, bo):
    """Host-side prep: transpose / retile / fold biases / cast to bf16+fp8."""
    x = np.asarray(x, np.float32)
    pos = np.asarray(pos_embeddings, np.float32)
    wqT = _to_fp8(np.asarray(Wq, np.float32).T.reshape(MT, P, D))
    wkT = _to_fp8(np.asarray(Wk, np.float32).T.reshape(MT, P, D))
    wpT = _to_fp8(np.asarray(Wp, np.float32).T.reshape(MT, P, D))
    wvT = _to_fp8(np.asarray(Wv, np.float32).T.reshape(MT, P, D))
    woT = _to_bf16(np.asarray(Wo, np.float32).T.reshape(MT, P, D))
    bq_t = np.asarray(bq, np.float32).reshape(MT, P, 1)
    bob = (
        np.asarray(bo, np.float32)
        + np.asarray(Wo, np.float32) @ np.asarray(bv, np.float32)
    ).reshape(MT, P, 1)

    in_maps = []
    for b in range(B):
        xT = _to_fp8(np.ascontiguousarray(x[b].T).reshape(MT, P, T))
        posT = _to_fp8(np.ascontiguousarray(pos[b].T).reshape(MT, P, T))
        in_maps.append(
            dict(
                xT=xT,
                posT=posT,
                wqT=wqT,
                wkT=wkT,
                wpT=wpT,
                wvT=wvT,
                woT=woT,
                bq=bq_t,
                bob=bob,
            )
        )
    return in_maps


def assemble_output(results):
    out = np.empty((B, T, D), np.float32)
    for b in range(B):
        out[b] = results[b]["outT"].reshape(D, T).T
    return out


def kernel(**inputs) -> np.ndarray:
    nc = build_nc()
    in_maps = make_in_maps(**inputs)
    res = run_bass_kernel_spmd(nc, in_maps, core_ids=list(range(N_CORES)))
    return assemble_output(res.results)


if __name__ == "__main__":
    import reference

    inputs = {k: np.asarray(v) for k, v in reference.setup_inputs().items()}
    got = kernel(**inputs)
    exp = np.asarray(reference.reference(**inputs))
    err = np.abs(got - exp)
    rel = np.linalg.norm(got - exp) / np.linalg.norm(exp)
    print("max abs err:", err.max(), "rel:", rel)
